# revision 6
# baseline (speedup 1.0000x reference)
"""Trainium2 Bass kernel for nn_ContrastiveLearning (NT-Xent over pairwise
symmetrized-KL of diagonal Gaussians).

Math (equivalent reformulation of the reference):
  total[i,j] := <s_i,u_j> + <u_i,s_j> - 2<w_i,m_j> - 2<m_i,w_j>
  with u=1/sigma, s=sigma+mu^2, w=mu/sigma, m=mu.
  Row-constant factors cancel in lf_i = -log2(num_i)+log2(den_i)+log2(cnt_i),
  so the device computes E[i,j] = exp(-0.25*(total[i,j] + q_j + pen_j) + C)
  where q_j = <mu_j^2, 1/sigma_j>, pen_j = BIG kills padding columns, the
  diagonal is killed by adding BIG*I via an extra matmul, and C keeps fp32
  exp in range.  num_i = sum_j E*eq[i,j], den_i = sum_j E.
  cnt, log2 and the final masked mean are O(N) host work.

Sharding: anchor rows are data-parallel over 8 cores; each core receives the
token axis rotated by core*N/8 (single SPMD program, static addressing).

Device dtypes (validated against the fp64 reference in simulation, final rel
err ~1e-5): h, W, pairwise features, E and the eq mask are bf16; the sigma
chain, q sums, exponent accumulation and num/den accumulators are fp32.
1/sigma uses the DVE reciprocal_approx_fast custom op (~18 bits) so the
scalar engine only ever touches one activation-table page (no reloads).
Phase-2 tiles for j-block g are issued right after group g's features so the
pairwise matmuls overlap the next group's projection/elementwise work.
"""
import numpy as np

EMBED_DIM = 128
H_DIM = 768
C_SHIFT = 40.0           # global exponent shift, cancels between num and den
BIG = 1024.0 * 1024.0    # diag/pad kill: exp(-0.25*BIG) == 0 in fp32
SQRT_BIG = 1024.0

_cache = {}


def _build(n_pad):
    import concourse.bass as bass
    import concourse.tile as tile
    from concourse import bacc, mybir

    f32 = mybir.dt.float32
    f32r = mybir.dt.float32r
    bf16 = mybir.dt.bfloat16
    AF = mybir.ActivationFunctionType
    ALU = mybir.AluOpType
    AX = mybir.AxisListType

    n_anchor = n_pad // 8            # anchors per core
    # non-uniform J-tiles: 512-wide groups plus a remainder group
    j_tiles = []
    off = 0
    while off < n_pad:
        w = min(512, n_pad - off)
        j_tiles.append((off, w))
        off += w
    n_groups = len(j_tiles)
    # I-tiles: 128-row chunks of this core's anchors plus a remainder
    i_tiles = []
    off = 0
    while off < n_anchor:
        w = min(128, n_anchor - off)
        i_tiles.append((off, w))
        off += w
    n_itiles = len(i_tiles)
    assert n_anchor <= 512, "diagonal assumed to land in j-group 0"

    nc = bacc.Bacc(None, target_bir_lowering=False, debug=False)
    ht_d = nc.declare_dram_parameter("ht", [H_DIM, n_pad], bf16, isOutput=False)
    wmu_d = nc.declare_dram_parameter("wmu", [128, 6, 128], bf16, isOutput=False)
    wsig_d = nc.declare_dram_parameter("wsig", [128, 6, 128], bf16, isOutput=False)
    bmu_d = nc.declare_dram_parameter("bmu", [EMBED_DIM], f32, isOutput=False)
    bsig_d = nc.declare_dram_parameter("bsig", [EMBED_DIM], f32, isOutput=False)
    eq_d = nc.declare_dram_parameter("eq", [n_anchor, n_pad], bf16, isOutput=False)
    pen_d = nc.declare_dram_parameter("pen", [n_pad], f32, isOutput=False)
    eqd_d = nc.declare_dram_parameter("eqd", [128, 128], bf16, isOutput=False)
    out_d = nc.declare_dram_parameter("out", [n_anchor, 2], f32, isOutput=True)

    with tile.TileContext(nc) as tc:
        with tc.tile_pool(name="const", bufs=1) as const, \
             tc.tile_pool(name="feat", bufs=1) as feat, \
             tc.tile_pool(name="chunk", bufs=2) as chunk, \
             tc.tile_pool(name="work", bufs=2) as work, \
             tc.tile_pool(name="acc", bufs=1) as accp, \
             tc.tile_pool(name="psP", bufs=2, space="PSUM") as psP, \
             tc.tile_pool(name="psQ", bufs=2, space="PSUM") as psQ, \
             tc.tile_pool(name="ps2", bufs=2, space="PSUM") as ps2p:

            # ---------------- small constants first ----------------
            wmu_t = const.tile([128, 6, 128], bf16)
            wsig_t = const.tile([128, 6, 128], bf16)
            nc.sync.dma_start(wmu_t[:], wmu_d[:, :, :])
            nc.sync.dma_start(wsig_t[:], wsig_d[:, :, :])
            bmu_t = const.tile([128, 1], f32)
            bsig_t = const.tile([128, 1], f32)
            eqd_t = const.tile([128, 128], bf16)
            pen_t = const.tile([1, n_pad], f32r)

            ones0 = const.tile([128, 1], f32)
            nc.vector.memset(ones0[:], 1.0)
            onescol_b = const.tile([128, 1], f32r)
            nc.vector.tensor_copy(onescol_b[:], ones0[:])
            ones1_r = const.tile([1, 1], f32r)
            nc.vector.tensor_copy(ones1_r[:], ones0[0:1, :])
            onesrow0 = const.tile([1, 128], f32)
            nc.vector.memset(onesrow0[:], 1.0)
            onesrow_b = const.tile([1, 128], f32r)
            nc.vector.tensor_copy(onesrow_b[:], onesrow0[:])
            cbias_t = const.tile([128, 1], f32)
            nc.vector.memset(cbias_t[:], C_SHIFT)

            # ---------------- persistent feature tensors ----------------
            mu_f = feat.tile([128, n_pad], bf16)     # mu
            u_f = feat.tile([128, n_pad], bf16)      # 1/sigma
            s_f = feat.tile([128, n_pad], bf16)      # sigma + mu^2
            wc_f = feat.tile([128, n_pad], bf16)     # mu/sigma
            qcol = feat.tile([1, n_pad], f32r)       # q_j + pen_j
            m2a = feat.tile([128, n_anchor], bf16)   # -2*mu      (anchors)
            w2a = feat.tile([128, n_anchor], bf16)   # -2*mu/sig  (anchors)
            eq_ts = [const.tile([iw, n_pad], bf16, tag=f"eq{t}", name=f"eq{t}")
                     for t, (ioff, iw) in enumerate(i_tiles)]

            num_sl = accp.tile([128, n_itiles * n_groups], f32)
            den_sl = accp.tile([128, n_itiles * n_groups], f32)

            ht_r = ht_d.rearrange("(a b) c -> b a c", b=128)  # [128, 6, n_pad]

            for g, (goff, gw) in enumerate(j_tiles):
                # ---------------- phase 1 for group g ----------------
                gs = slice(goff, goff + gw)
                hg = []
                for kk in range(6):
                    hraw = chunk.tile([128, 512], bf16, tag=f"hraw{kk}")
                    nc.sync.dma_start(hraw[:, :gw], ht_r[:, kk, gs])
                    hk = chunk.tile([128, 512], bf16, tag=f"hg{kk}")
                    if kk % 2 == 0:
                        nc.vector.tensor_scalar_max(hk[:, :gw], hraw[:, :gw], 0.0)
                    else:
                        nc.scalar.activation(hk[:, :gw], hraw[:, :gw], AF.Relu)
                    hg.append(hk)
                if g == 0:
                    # phase-2 / elementwise-only inputs: issued after group-0's
                    # ht pieces so they never delay the first relu+matmuls, but
                    # in program order before every consumer
                    for t, (ioff, iw) in enumerate(i_tiles):
                        nc.sync.dma_start(eq_ts[t][:], eq_d[ioff:ioff + iw, :])
                    nc.sync.dma_start(bmu_t[:], bmu_d.rearrange("(p o) -> p o", o=1))
                    nc.sync.dma_start(bsig_t[:], bsig_d.rearrange("(p o) -> p o", o=1))
                    nc.sync.dma_start(pen_t[:], pen_d.rearrange("(o n) -> o n", o=1).bitcast(f32r))
                    nc.sync.dma_start(eqd_t[:], eqd_d[:, :])

                ps_mu = psP.tile([128, 512], f32, tag="mu")
                ps_z = psP.tile([128, 512], f32, tag="z")
                for kk in range(6):
                    nc.tensor.matmul(ps_mu[:, :gw], wmu_t[:, kk, :], hg[kk][:, :gw],
                                     start=(kk == 0), stop=(kk == 5))
                for kk in range(6):
                    nc.tensor.matmul(ps_z[:, :gw], wsig_t[:, kk, :], hg[kk][:, :gw],
                                     start=(kk == 0), stop=(kk == 5))

                nc.scalar.activation(mu_f[:, gs], ps_mu[:, :gw], AF.Identity, bias=bmu_t[:])
                # sigma = exp(min(z,0)) + 1e-14 + relu(z)   (elu(z)+1)
                zm = work.tile([128, 512], f32, tag="zm")
                nc.vector.tensor_scalar(zm[:, :gw], ps_z[:, :gw], bsig_t[:], 0.0, ALU.add, ALU.min)
                zp = work.tile([128, 512], f32, tag="zp")
                nc.vector.tensor_scalar(zp[:, :gw], ps_z[:, :gw], bsig_t[:], 0.0, ALU.add, ALU.max)
                e1 = work.tile([128, 512], f32, tag="e1")
                nc.scalar.activation(e1[:, :gw], zm[:, :gw], AF.Exp)
                sig = work.tile([128, 512], f32, tag="sig")
                nc.vector.scalar_tensor_tensor(sig[:, :gw], e1[:, :gw], 1e-14, zp[:, :gw],
                                               ALU.add, ALU.add)
                u_w = work.tile([128, 512], f32, tag="u_w")
                nc.vector.reciprocal_approx_fast(u_w[:, :gw], sig[:, :gw])
                nc.scalar.activation(u_f[:, gs], u_w[:, :gw], AF.Identity)
                psq = work.tile([128, 512], f32, tag="psq")
                nc.vector.tensor_mul(psq[:, :gw], mu_f[:, gs], mu_f[:, gs])
                nc.vector.tensor_add(s_f[:, gs], psq[:, :gw], sig[:, :gw])
                nc.vector.tensor_mul(wc_f[:, gs], mu_f[:, gs], u_f[:, gs])
                pu = work.tile([128, 512], f32r, tag="pu")
                nc.vector.tensor_mul(pu[:, :gw], mu_f[:, gs], wc_f[:, gs])
                ps_q = psQ.tile([1, 512], f32, tag="q")
                nc.tensor.matmul(ps_q[:, :gw], onescol_b[:], pu[:, :gw],
                                 start=True, stop=True)
                # pen add rides the DVE copy instead of a second matmul
                nc.vector.tensor_add(qcol[:, gs], ps_q[:, :gw], pen_t[:, gs])

                if g == 0:
                    # anchor stationaries carry the -2 of the cross terms
                    nc.vector.tensor_scalar_mul(m2a[:], mu_f[:, 0:n_anchor], -2.0)
                    nc.vector.tensor_scalar_mul(w2a[:], wc_f[:, 0:n_anchor], -2.0)

                # ---------------- phase 2 tiles with jt == g ----------------
                jsl = gs
                for t, (ioff, iw) in enumerate(i_tiles):
                    isl = slice(ioff, ioff + iw)
                    ps2 = ps2p.tile([128, 512], f32, tag="p2")
                    nc.tensor.matmul(ps2[:iw, :gw], s_f[:, isl], u_f[:, jsl],
                                     start=True, stop=False)
                    nc.tensor.matmul(ps2[:iw, :gw], u_f[:, isl], s_f[:, jsl],
                                     start=False, stop=False)
                    nc.tensor.matmul(ps2[:iw, :gw], w2a[:, isl], mu_f[:, jsl],
                                     start=False, stop=False)
                    nc.tensor.matmul(ps2[:iw, :gw], m2a[:, isl], wc_f[:, jsl],
                                     start=False, stop=False)
                    last = (g != 0)
                    nc.tensor.matmul(ps2[:iw, :gw], onesrow_b[:, :iw], qcol[:, jsl],
                                     start=False, stop=last)
                    if g == 0:
                        # diagonal kill: all diag cols of this core's anchors
                        # fall inside j-group 0 (n_anchor <= 512)
                        nc.tensor.matmul(ps2[:iw, ioff:ioff + iw],
                                         eqd_t[:, :iw], eqd_t[:, :iw],
                                         start=False, stop=True)
                    e_t = work.tile([128, 512], bf16, tag="E", bufs=3)
                    col = t * n_groups + g
                    nc.scalar.activation(e_t[:iw, :gw], ps2[:iw, :gw], AF.Exp, scale=-0.25,
                                         bias=cbias_t[:iw, :],
                                         accum_out=den_sl[:iw, col:col + 1])
                    msk = work.tile([128, 512], bf16, tag="msk", bufs=3)
                    nc.vector.scalar_tensor_tensor(msk[:iw, :gw], e_t[:iw, :gw], 1.0,
                                                   eq_ts[t][:, jsl],
                                                   ALU.mult, ALU.mult,
                                                   accum_out=num_sl[:iw, col:col + 1])

            nd = accp.tile([128, n_itiles, 2], f32)
            for t, (ioff, iw) in enumerate(i_tiles):
                nc.vector.tensor_reduce(nd[:iw, t, 0:1],
                                        num_sl[:iw, t * n_groups:(t + 1) * n_groups],
                                        AX.X, ALU.add)
                nc.vector.tensor_reduce(nd[:iw, t, 1:2],
                                        den_sl[:iw, t * n_groups:(t + 1) * n_groups],
                                        AX.X, ALU.add)
                nc.sync.dma_start(
                    out_d[ioff:ioff + iw, :], nd[:iw, t, :])

    nc.compile()
    return nc


def _prep(ent_embeddings, ent_type_ids, ent_mask, W_mu, b_mu, W_sigma, b_sigma):
    """Host-side compaction / layout. Returns (in_maps, meta) or (None, scalar)."""
    import ml_dtypes
    bf = ml_dtypes.bfloat16

    emb = np.ascontiguousarray(np.asarray(ent_embeddings, dtype=np.float32)).reshape(-1, H_DIM)
    labels = np.asarray(ent_type_ids).reshape(-1).astype(np.int64)
    mask = np.asarray(ent_mask).reshape(-1).astype(np.int64)
    W_mu = np.asarray(W_mu, dtype=np.float32)
    W_sigma = np.asarray(W_sigma, dtype=np.float32)
    b_mu = np.ascontiguousarray(np.asarray(b_mu, dtype=np.float32))
    b_sigma = np.ascontiguousarray(np.asarray(b_sigma, dtype=np.float32))

    valid = (mask == 1) & (labels >= 0)
    vidx = np.nonzero(valid)[0]
    n_v = len(vidx)
    if n_v == 0:
        return None, np.float32(0.0)

    # tightest multiple of 64 (>=1024) that holds all valid tokens; keeps
    # n_anchor = n_pad/8 integral and j-tail widths DMA-friendly
    n_pad = max(1024, ((n_v + 63) // 64) * 64)
    assert n_v <= n_pad <= 4096, f"too many valid tokens: {n_v}"
    n_anchor = n_pad // 8

    embT = np.zeros((H_DIM, n_pad), dtype=bf)
    embT[:, :n_v] = emb[vidx].T.astype(bf)
    labc_v = np.full(n_pad, -1.0, dtype=np.float32)
    labc_v[:n_v] = labels[vidx].astype(np.float32)
    pen_v = np.full(n_pad, BIG, dtype=np.float32)
    pen_v[:n_v] = 0.0
    # [H,D] -> [128 part, 6 piece, 128 D] with H row = piece*128 + part
    wmu_b = np.ascontiguousarray(
        W_mu.astype(bf).reshape(6, 128, 128).transpose(1, 0, 2))
    wsig_b = np.ascontiguousarray(
        W_sigma.astype(bf).reshape(6, 128, 128).transpose(1, 0, 2))
    eqd = (np.eye(128, dtype=np.float32) * SQRT_BIG).astype(bf)

    in_maps = []
    for c in range(8):
        r = c * n_anchor
        labr = np.roll(labc_v, -r)
        eq = (labr[:n_anchor, None] == labr[None, :]).astype(bf)
        in_maps.append({
            "ht": np.ascontiguousarray(np.roll(embT, -r, axis=1)),
            "wmu": wmu_b, "wsig": wsig_b, "bmu": b_mu, "bsig": b_sigma,
            "eq": eq,
            "pen": np.roll(pen_v, -r),
            "eqd": eqd,
        })
    meta = (labels, vidx, n_v, n_pad, n_anchor)
    return in_maps, meta


def _epilogue(res, meta):
    labels, vidx, n_v, n_pad, n_anchor = meta
    num = np.empty(n_pad, dtype=np.float32)
    den = np.empty(n_pad, dtype=np.float32)
    for c in range(8):
        nd = res.results[c]["out"]
        rows = (np.arange(n_anchor) + c * n_anchor) % n_pad
        num[rows] = nd[:, 0]
        den[rows] = nd[:, 1]

    labs = labels[vidx]
    hist = np.bincount(labs, minlength=int(labs.max()) + 1)
    cnt = (hist[labs] - 1).astype(np.float64)
    sel = cnt > 0
    n_sel = max(sel.sum(), 1)
    num_v = num[:n_v].astype(np.float64)
    den_v = den[:n_v].astype(np.float64)
    safe_num = np.where(sel, num_v, 1.0)
    safe_den = np.where(sel, den_v, 1.0)
    safe_cnt = np.where(sel, cnt, 1.0)
    lf = (np.log(safe_den) - np.log(safe_num)) / np.log(2.0) + np.log2(safe_cnt)
    total = np.sum(np.where(sel, lf, 0.0)) / n_sel
    return np.float32(total)


def kernel(ent_embeddings, ent_type_ids, ent_mask, W_mu, b_mu, W_sigma, b_sigma):
    from concourse.bass_utils import run_bass_kernel_spmd

    in_maps, meta = _prep(ent_embeddings, ent_type_ids, ent_mask,
                          W_mu, b_mu, W_sigma, b_sigma)
    if in_maps is None:
        return meta
    n_pad = meta[3]
    if n_pad not in _cache:
        _cache[n_pad] = _build(n_pad)
    nc = _cache[n_pad]
    res = run_bass_kernel_spmd(nc, in_maps, list(range(8)))
    return _epilogue(res, meta)

